# revision 27
# baseline (speedup 1.0000x reference)
"""Trainium2 Bass kernel for nn_Attention_61907658605177.

Self-attention where q == k == v, each equal to the input x reinterpreted as
[H=16, B=2, S=2048, hd=64].  Output is att.swapaxes(1,2).reshape(-1, 1024).

Sharding: the 32 independent (h, b) pairs are split 4-per-core across the
8 NeuronCores (pure data parallelism, no collectives).  Each pair's slice of
x's flat buffer is contiguous, so core i receives x.reshape(32, 2048, 64)[4i:4i+4].

Per-core algorithm (per pair, S=2048, hd=64):
  - scores = (qT.T @ qT) / 8 on TensorE in fp16, one 128-row q-tile at a
    time, split into two half-tiles computed concurrently in the two K=64
    row-groups of the PE array (qT duplicated across both partition halves).
  - exp on ScalarE straight out of PSUM: E = exp(scores/8 - 8) in fp16.
    The PSUM scores ring has 3 half-tile slots; consecutive units rotate
    slots so exp(h) overlaps mm1(h+1).  Units landing in slots (0,1) are
    exp'd with a single merged [128, 2048] instruction.
  - att = E_unnorm @ v_ext via PSUM-accumulated matmuls using E's symmetry
    (q == k): att[qi,:] = sum_c E[c-chunk, qi-tile]^T @ v[c-chunk].  v_ext
    carries a ones column, so column 64 of the accumulator is the softmax
    denominator for each row (again by symmetry, row sums == column sums).
  - VectorE computes 1/denominator and scales while evacuating PSUM; DMA out.
"""

import sys

if "/opt/trn_rl_repo" not in sys.path:
    sys.path.insert(0, "/opt/trn_rl_repo")

import numpy as np

import concourse.bass as bass
import concourse.mybir as mybir
import concourse.tile as tile
from concourse.bass_utils import run_bass_kernel_spmd
from concourse.masks import make_identity

F32 = mybir.dt.float32
F16 = mybir.dt.float16

H, B, S, E_DIM, HD = 16, 2, 2048, 1024, 64
N_CORES = 8
PAIRS = 4            # (h, b) pairs per core
T = S // 128         # 16 q-tiles per pair
C = S // 128         # 16 ki-chunks per pair
U = 2 * T            # 32 half-tile units per pair
SCALE = 0.125        # 1/sqrt(DK) with DK=64
EXP_BIAS = -8.0      # constant shift inside exp; cancels in softmax

ts = bass.ts


def _apply_tile_drain_patch():
    """walrus in this toolchain rejects instructions carrying too many sync
    commands; re-emit the kernel-tail drain's waits as standalone ops."""
    from bass_rust import ScopedClock

    def _drain_and_barrier_split(self, tick_clock, wait_clock):
        nc = self.nc
        drain = nc.sync.drain()
        wait_clock.add_sem_waits(
            drain.ins, ScopedClock({None: tick_clock.global_clock})
        )
        si = drain.ins.sync_info
        waits = list(si.on_wait or []) if si is not None else []
        if len(waits) > 1:
            si.on_wait = []
            name_to_handle = {h.name: h for h in self.sems.allocated().values()}
            for w in waits:
                nc.sync.wait_ge(name_to_handle[w.ant_name], w.wait_value)
            nc.sync.drain()
        nc.all_engine_barrier()
        popped = nc._tile_sem_poison_stack.pop()
        assert popped is self._sem_poison
        nc.clear_and_free_semaphores(list(self.sems.allocated().values()))
        nc.all_engine_barrier()

    tile.TileContext._drain_and_barrier = _drain_and_barrier_split


_DMA_LIKE = (
    mybir.InstDMACopy,
    mybir.InstDmaTransposeAnt,
    mybir.InstDMA,
    mybir.InstCollectiveCompute,
    mybir.InstDrain,
    mybir.InstNoOp,
)


def _split_sync_waits(nc):
    """walrus rejects instructions with more than ~2 total sync commands
    (waits + updates; DMA pseudos tolerate only 1 wait).  Move overflow waits
    onto preceding same-engine NoOps, one wait each."""
    nsplit = 0
    for f in nc.m.functions:
        for b in f.blocks:
            live = b.instructions
            insts = list(live)
            out_insts = []
            changed = False
            for inst in insts:
                si = getattr(inst, "sync_info", None)
                waits = list(si.on_wait) if (si is not None and si.on_wait) else []
                nupd = len(si.on_update) if (si is not None and si.on_update) else 0
                maxw = max(0, 2 - nupd)
                if isinstance(inst, _DMA_LIKE):
                    maxw = min(maxw, 1)
                if len(waits) > maxw:
                    si.on_wait = waits[-maxw:] if maxw > 0 else []
                    for w in (waits[:-maxw] if maxw > 0 else waits):
                        nsplit += 1
                        nop = mybir.InstNoOp(
                            name=f"wsplit-{nsplit}-{inst.name}",
                            engine=inst.engine,
                            sync_info=mybir.SyncInfo(on_wait=[w], on_update=[]),
                        )
                        out_insts.append(nop)
                    changed = True
                out_insts.append(inst)
            if changed:
                live[:] = out_insts
    return nsplit


def build_kernel(loop_reps: int = 1):
    _apply_tile_drain_patch()
    nc = bass.Bass()
    x = nc.declare_dram_parameter("x", [PAIRS, S, HD], F32, isOutput=False)
    out = nc.declare_dram_parameter("out", [PAIRS, S, HD], F32, isOutput=True)

    with tile.TileContext(nc) as tc:
        with (
            tc.tile_pool(name="singles", bufs=1) as singles,
            tc.tile_pool(name="qn", bufs=4) as qn_pool,
            tc.tile_pool(name="v", bufs=3) as v_pool,
            tc.tile_pool(name="qT2", bufs=2) as qT2_pool,
            tc.tile_pool(name="ebuf", bufs=2) as e_pool,
            tc.tile_pool(name="sums", bufs=2) as sums_pool,
            tc.tile_pool(name="outsb", bufs=4) as out_pool,
            tc.tile_pool(name="ringA", bufs=1, space="PSUM") as ringA_pool,
            tc.tile_pool(name="ringB", bufs=1, space="PSUM") as ringB_pool,
            tc.tile_pool(name="attp0", bufs=1, space="PSUM") as att0_pool,
            tc.tile_pool(name="attp1", bufs=1, space="PSUM") as att1_pool,
        ):
            ident = singles.tile([128, 128], F16)
            make_identity(nc, ident)
            bias_tile = singles.tile([128, 1], F32)
            nc.vector.memset(bias_tile, EXP_BIAS)
            # dummy exp: pulls the ACT exp table load (~2.7us) off the
            # critical path by overlapping it with the first input DMA
            warm = singles.tile([128, 1], F32)
            nc.scalar.activation(
                warm[:], bias_tile[:], mybir.ActivationFunctionType.Exp
            )

            # PSUM map (8 banks): ringA 4, ringB 2, att0 1, att1 1.
            # Tile's WAR tracking is tensor-granular, so the two merged-exp
            # slots and the single-exp slot live in separate tensors; that way
            # an mm1 only waits on the exp that actually read its slot.
            # Transposes stage through ringB (viewed as f16).
            ringA = ringA_pool.tile([128, 2, 1024], F32, name="ringA")
            ringB = ringB_pool.tile([128, 1024], F32, name="ringB")
            att01 = (
                att0_pool.tile([128, HD + 1], F32, name="att0"),  # mm2 accum ping-pong,
                att1_pool.tile([128, HD + 1], F32, name="att1"),  # separate banks
            )
            qtp_view = ringB[0:64, :].bitcast(F16)       # [64, 2048] staging

            st = [dict() for _ in range(PAIRS)]

            def dma_in(p, half=None):
                # p-major layout: partition pp holds input rows 16*pp..16*pp+15,
                # i.e. q-tile t = rows {16*pp + t}. Attention is row-order
                # agnostic, and the output DMA's (pp, t)-major DRAM order maps
                # back to exactly the natural row order (16*pp + t).
                if half in (None, 0):
                    st[p]["qn"] = qn_pool.tile(
                        [128, T, HD], F32, tag="qn", name="qn"
                    )
                qn = st[p]["qn"]
                xr = x[p].rearrange("(pp j) d -> pp j d", j=16)
                if half is None:
                    nc.sync.dma_start(out=qn[:], in_=xr)
                elif half == 0:
                    nc.sync.dma_start(out=qn[:, 0:8, :], in_=xr[:, 0:8, :])
                else:
                    nc.sync.dma_start(out=qn[:, 8:16, :], in_=xr[:, 8:16, :])

            def cast(p, half=None):
                if half in (None, 0):
                    v = v_pool.tile([128, C, HD + 1], F16, tag="v", name="v")
                    nc.vector.memset(v[:, :, HD : HD + 1], 1.0)
                    st[p]["v"] = v
                v = st[p]["v"]
                if half is None:
                    nc.vector.tensor_copy(v[:, :, 0:HD], st[p]["qn"][:])
                elif half == 0:
                    nc.vector.tensor_copy(v[:, 0:8, 0:HD], st[p]["qn"][:, 0:8, :])
                else:
                    nc.vector.tensor_copy(v[:, 8:16, 0:HD], st[p]["qn"][:, 8:16, :])
                if half in (None, 0):
                    st[p]["qT2"] = qT2_pool.tile([128, S], F16, tag="qT2", name="qT2")
                    st[p]["recips"] = sums_pool.tile([128, T], F32, tag="recips", name="recips")
                    st[p]["outsb"] = out_pool.tile([128, T, HD], F32, tag="outsb", name="outsb")

            def transpose_chunk(p, k, view=None):
                # four [128,64] -> [64,128] transposes staged in a PSUM bank
                # (ringB by default), then copied into the low half of qT2
                v = st[p]["v"]
                qT2 = st[p]["qT2"]
                view = qtp_view if view is None else view
                for i, j in enumerate(range(4 * k, 4 * k + 4)):
                    nc.tensor.transpose(
                        view[:, ts(i, 128)], v[:, j, 0:HD], ident[:]
                    )
                nc.vector.tensor_copy(qT2[0:64, ts(k, 512)], view[:, 0:512])

            def dup_chunk(p, k):
                # duplicate into the upper partition half SBUF->SBUF (4x DVE
                # mode, and it keeps the PSUM staging window short)
                qT2 = st[p]["qT2"]
                nc.vector.tensor_copy(
                    qT2[64:128, ts(k, 512)], qT2[0:64, ts(k, 512)]
                )

            def mm1(p, u, s, rg=None):
                # half-tile unit u: q-tile t = u//2, ki columns (u%2)*1024..,
                # written to ring slot s; rg picks the PE row-group (0: array
                # rows 0-63 fed from qT2[0:64], 1: rows 64-127 from the upper
                # duplicate) so adjacent units can run concurrently
                qT2 = st[p]["qT2"]
                t, half = divmod(u, 2)
                rg = half if rg is None else rg
                dst = ringB[:] if s == 2 else ringA[:, s, :]
                rows = slice(0, 64) if rg == 0 else slice(64, 128)
                lhsT = qT2[rows, ts(t, 128)]
                base = half * 1024
                for j in range(2):
                    nc.tensor.matmul(
                        dst[:, ts(j, 512)],
                        lhsT,
                        qT2[rows, base + j * 512 : base + (j + 1) * 512],
                    )

            def _e32(p):
                return st[p]["ebuf"].rearrange("pp c s -> pp (c s)").rearrange(
                    "pp (u k) -> pp u k", k=1024
                )

            def exp_merged(p, ua, ub):
                # one [128, 2048] exp over ringA slots 0,1 -> E units ua, ub
                e32 = _e32(p)
                if ub == ua + 1:
                    dst = e32[:, ua : ua + 2, :]
                else:
                    dst = bass.AP(
                        tensor=e32.tensor,
                        offset=e32.offset + ua * 1024,
                        ap=[list(e32.ap[0]), [(ub - ua) * 1024, 2], [1, 1024]],
                    )
                nc.scalar.activation(
                    dst,
                    ringA[:],
                    mybir.ActivationFunctionType.Exp,
                    bias=bias_tile[:],
                    scale=SCALE,
                )

            def exp_single(p, u):
                nc.scalar.activation(
                    _e32(p)[:, u, :],
                    ringB[:],
                    mybir.ActivationFunctionType.Exp,
                    bias=bias_tile[:],
                    scale=SCALE,
                )

            def mm2norm(p, t, tail=False):
                ebuf, v = st[p]["ebuf"], st[p]["v"]
                if tail:
                    # 3-way rotation over decoupled tensors so the DVE
                    # normalize of tile t-1 overlaps the matmuls of tile t
                    slot = (att01[0][:], att01[1][:], ringB[:, 0 : HD + 1])[t % 3]
                else:
                    slot = att01[t % 2][:]
                for c in range(C):
                    nc.tensor.matmul(
                        slot,
                        ebuf[:, c, ts(t, 128)],
                        v[:, c, :],
                        start=(c == 0),
                        stop=(c == C - 1),
                    )
                rc = st[p]["recips"]
                nc.vector.reciprocal(rc[:, t : t + 1], slot[:, HD : HD + 1])
                nc.vector.tensor_scalar_mul(
                    st[p]["outsb"][:, t, :], slot[:, 0:HD], rc[:, t : t + 1]
                )

            def dma_out(p, quarter=None):
                odram = out[p].rearrange("(pp t) d -> pp t d", t=16)
                if quarter is None:
                    nc.sync.dma_start(out=odram, in_=st[p]["outsb"][:])
                else:
                    q4 = ts(quarter, 4)
                    nc.sync.dma_start(
                        out=odram[:, q4, :], in_=st[p]["outsb"][:, q4, :]
                    )

            def alloc_e(p):
                st[p]["ebuf"] = e_pool.tile([128, C, S], F16, tag="ebuf", name="ebuf")

            def prologue_step(p, h):
                """Spread pair-(p) prologue work across the previous pair's
                unit steps h (called with h = 0..U-1 of the previous pair).
                Transpose chunks are emitted right after a merged exp (h%3==1)
                so the slot-2 borrow lands in slot 2's idle window."""
                if h == 0:
                    dma_in(p)
                elif h == 2:
                    cast(p)
                elif h in (12, 16, 20, 24):
                    transpose_chunk(p, (h - 12) // 4)
                    dup_chunk(p, (h - 12) // 4)
                    if h == 24:
                        alloc_e(p)

            def emit_body():
                # pair-0 prologue, interleaved: chunks alternate between
                # ringB and ringA staging so transposes overlap copies, and
                # the first two mm1 units slot in as their inputs appear
                ra0 = ringA[0:64, 0, :].bitcast(F16)
                dma_in(0, half=0)
                cast(0, half=0)
                dma_in(0, half=1)
                transpose_chunk(0, 0)
                transpose_chunk(0, 1, view=ra0)
                alloc_e(0)
                mm1(0, 0, 0, rg=0)   # unit 0 (tile 0, ki 0-1023)
                mm1(0, 2, 1, rg=0)   # unit 2 (tile 1, ki 0-1023)
                exp_merged(0, 0, 2)
                cast(0, half=1)
                transpose_chunk(0, 2)
                transpose_chunk(0, 3)
                dma_in(1)  # pair-1 prologue_step(i=0) equivalent
                for k in range(4):
                    dup_chunk(0, k)

                # unit processing order per pair: pair 0 and the last pair run
                # all even units (ki columns 0-1023 of every tile) before the
                # odd ones.  For pair 0 that lets compute start after only two
                # transpose chunks; for the last pair it lets the tail's mm2
                # for q-tiles 0-7 (which reads only even units) start while
                # the odd units are still being exp'd.
                orders = [list(range(U)) for _ in range(PAIRS)]
                orders[0] = list(range(0, U, 2)) + list(range(1, U, 2))
                orders[PAIRS - 1] = list(range(0, U, 2)) + list(range(1, U, 2))

                def aux(p, i):
                    # non-critical work scheduled into unit step i of pair p
                    if p >= 1 and 4 <= i <= 19:
                        mm2norm(p - 1, i - 4)
                        if i == 19:
                            dma_out(p - 1)
                    if p == PAIRS - 1 and 20 <= i <= 27:
                        mm2norm(p, i - 20)
                        if i in (23, 27):
                            dma_out(p, quarter=(i - 23) // 4)
                    if p + 1 < PAIRS:
                        prologue_step(p + 1, i)

                for p in range(PAIRS):
                    order = orders[p]
                    for i in range(2 if p == 0 else 0, U):
                        mm1(p, order[i], i % 3, rg=i % 2)
                        if i % 3 == 1:
                            exp_merged(p, order[i - 1], order[i])
                            aux(p, i - 1)
                            aux(p, i)
                        elif i % 3 == 2:
                            exp_single(p, order[i])
                            aux(p, i)

                p = PAIRS - 1
                for t in range(8, T):
                    mm2norm(p, t, tail=True)
                    if t % 4 == 3:
                        dma_out(p, quarter=t // 4)

            if loop_reps > 1:
                with tc.For_i(0, loop_reps, 1):
                    emit_body()
            else:
                emit_body()

    _split_sync_waits(nc)
    return nc


_NC_CACHE = None


def kernel(x: np.ndarray) -> np.ndarray:
    global _NC_CACHE
    if _NC_CACHE is None:
        _NC_CACHE = build_kernel()
    nc = _NC_CACHE

    x = np.asarray(x, dtype=np.float32)
    xr = np.reshape(x, (H * B, S, HD))  # flat-buffer reinterpret: pair = h*B + b
    in_maps = [
        {"x": np.ascontiguousarray(xr[i * PAIRS : (i + 1) * PAIRS])}
        for i in range(N_CORES)
    ]
    res = run_bass_kernel_spmd(nc, in_maps, core_ids=list(range(N_CORES)))
    att = np.concatenate([res.results[i]["out"] for i in range(N_CORES)], axis=0)
    att = att.reshape(H, B, S, HD).swapaxes(1, 2).reshape(-1, E_DIM)
    return np.ascontiguousarray(att.astype(np.float32))


# revision 35
# speedup vs baseline: 1.0010x; 1.0010x over previous
"""Trainium2 Bass kernel for nn_Attention_61907658605177.

Self-attention where q == k == v, each equal to the input x reinterpreted as
[H=16, B=2, S=2048, hd=64].  Output is att.swapaxes(1,2).reshape(-1, 1024).

Sharding: the 32 independent (h, b) pairs are split 4-per-core across the
8 NeuronCores (pure data parallelism, no collectives).  Each pair's slice of
x's flat buffer is contiguous, so core i receives x.reshape(32, 2048, 64)[4i:4i+4].

Per-core algorithm (per pair, S=2048, hd=64):
  - scores = (qT.T @ qT) / 8 on TensorE in fp16, one 128-row q-tile at a
    time, split into two half-tiles computed concurrently in the two K=64
    row-groups of the PE array (qT duplicated across both partition halves).
  - exp on ScalarE straight out of PSUM: E = exp(scores/8 - 8) in fp16.
    The PSUM scores ring has 3 half-tile slots; consecutive units rotate
    slots so exp(h) overlaps mm1(h+1).  Units landing in slots (0,1) are
    exp'd with a single merged [128, 2048] instruction.
  - att = E_unnorm @ v_ext via PSUM-accumulated matmuls using E's symmetry
    (q == k): att[qi,:] = sum_c E[c-chunk, qi-tile]^T @ v[c-chunk].  v_ext
    carries a ones column, so column 64 of the accumulator is the softmax
    denominator for each row (again by symmetry, row sums == column sums).
  - VectorE computes 1/denominator and scales while evacuating PSUM; DMA out.
"""

import sys

if "/opt/trn_rl_repo" not in sys.path:
    sys.path.insert(0, "/opt/trn_rl_repo")

import numpy as np

import concourse.bass as bass
import concourse.mybir as mybir
import concourse.tile as tile
from concourse.bass_utils import run_bass_kernel_spmd
from concourse.masks import make_identity

F32 = mybir.dt.float32
F16 = mybir.dt.float16

H, B, S, E_DIM, HD = 16, 2, 2048, 1024, 64
N_CORES = 8
PAIRS = 4            # (h, b) pairs per core
T = S // 128         # 16 q-tiles per pair
C = S // 128         # 16 ki-chunks per pair
U = 2 * T            # 32 half-tile units per pair
SCALE = 0.125        # 1/sqrt(DK) with DK=64
EXP_BIAS = -8.0      # constant shift inside exp; cancels in softmax

ts = bass.ts


def _apply_tile_drain_patch():
    """walrus in this toolchain rejects instructions carrying too many sync
    commands; re-emit the kernel-tail drain's waits as standalone ops."""
    from bass_rust import ScopedClock

    def _drain_and_barrier_split(self, tick_clock, wait_clock):
        nc = self.nc
        drain = nc.sync.drain()
        wait_clock.add_sem_waits(
            drain.ins, ScopedClock({None: tick_clock.global_clock})
        )
        si = drain.ins.sync_info
        waits = list(si.on_wait or []) if si is not None else []
        if len(waits) > 1:
            si.on_wait = []
            name_to_handle = {h.name: h for h in self.sems.allocated().values()}
            for w in waits:
                nc.sync.wait_ge(name_to_handle[w.ant_name], w.wait_value)
            nc.sync.drain()
        nc.all_engine_barrier()
        popped = nc._tile_sem_poison_stack.pop()
        assert popped is self._sem_poison
        nc.clear_and_free_semaphores(list(self.sems.allocated().values()))
        nc.all_engine_barrier()

    tile.TileContext._drain_and_barrier = _drain_and_barrier_split


_DMA_LIKE = (
    mybir.InstDMACopy,
    mybir.InstDmaTransposeAnt,
    mybir.InstDMA,
    mybir.InstCollectiveCompute,
    mybir.InstDrain,
    mybir.InstNoOp,
)


def _split_sync_waits(nc):
    """walrus rejects instructions with more than ~2 total sync commands
    (waits + updates; DMA pseudos tolerate only 1 wait).  Move overflow waits
    onto preceding same-engine NoOps, one wait each."""
    nsplit = 0
    for f in nc.m.functions:
        for b in f.blocks:
            live = b.instructions
            insts = list(live)
            out_insts = []
            changed = False
            for inst in insts:
                si = getattr(inst, "sync_info", None)
                waits = list(si.on_wait) if (si is not None and si.on_wait) else []
                nupd = len(si.on_update) if (si is not None and si.on_update) else 0
                maxw = max(0, 2 - nupd)
                if isinstance(inst, _DMA_LIKE):
                    maxw = min(maxw, 1)
                if len(waits) > maxw:
                    si.on_wait = waits[-maxw:] if maxw > 0 else []
                    for w in (waits[:-maxw] if maxw > 0 else waits):
                        nsplit += 1
                        nop = mybir.InstNoOp(
                            name=f"wsplit-{nsplit}-{inst.name}",
                            engine=inst.engine,
                            sync_info=mybir.SyncInfo(on_wait=[w], on_update=[]),
                        )
                        out_insts.append(nop)
                    changed = True
                out_insts.append(inst)
            if changed:
                live[:] = out_insts
    return nsplit


def build_kernel(loop_reps: int = 1):
    _apply_tile_drain_patch()
    nc = bass.Bass()
    x = nc.declare_dram_parameter("x", [PAIRS, S, HD], F32, isOutput=False)
    out = nc.declare_dram_parameter("out", [PAIRS, S, HD], F32, isOutput=True)

    with tile.TileContext(nc) as tc:
        with (
            tc.tile_pool(name="singles", bufs=1) as singles,
            tc.tile_pool(name="qn", bufs=4) as qn_pool,
            tc.tile_pool(name="v", bufs=3) as v_pool,
            tc.tile_pool(name="qT2", bufs=2) as qT2_pool,
            tc.tile_pool(name="ebuf", bufs=2) as e_pool,
            tc.tile_pool(name="sums", bufs=2) as sums_pool,
            tc.tile_pool(name="outsb", bufs=4) as out_pool,
            tc.tile_pool(name="ringA", bufs=1, space="PSUM") as ringA_pool,
            tc.tile_pool(name="ringB", bufs=1, space="PSUM") as ringB_pool,
            tc.tile_pool(name="attp0", bufs=1, space="PSUM") as att0_pool,
            tc.tile_pool(name="attp1", bufs=1, space="PSUM") as att1_pool,
        ):
            ident = singles.tile([128, 128], F16)
            make_identity(nc, ident)
            bias_tile = singles.tile([128, 1], F32)
            nc.vector.memset(bias_tile, EXP_BIAS)
            # dummy exp: pulls the ACT exp table load (~2.7us) off the
            # critical path by overlapping it with the first input DMA
            warm = singles.tile([128, 1], F32)
            nc.scalar.activation(
                warm[:], bias_tile[:], mybir.ActivationFunctionType.Exp
            )

            # PSUM map (8 banks): ringA 4, ringB 2, att0 1, att1 1.
            # Tile's WAR tracking is tensor-granular, so the two merged-exp
            # slots and the single-exp slot live in separate tensors; that way
            # an mm1 only waits on the exp that actually read its slot.
            # Transposes stage through ringB (viewed as f16).
            ringA = ringA_pool.tile([128, 2, 1024], F32, name="ringA")
            ringB = ringB_pool.tile([128, 1024], F32, name="ringB")
            att01 = (
                att0_pool.tile([128, HD + 1], F32, name="att0"),  # mm2 accum ping-pong,
                att1_pool.tile([128, HD + 1], F32, name="att1"),  # separate banks
            )
            qtp_view = ringB[0:64, :].bitcast(F16)       # [64, 2048] staging

            st = [dict() for _ in range(PAIRS)]

            def dma_in(p, half=None):
                # p-major layout: partition pp holds input rows 16*pp..16*pp+15,
                # i.e. q-tile t = rows {16*pp + t}. Attention is row-order
                # agnostic, and the output DMA's (pp, t)-major DRAM order maps
                # back to exactly the natural row order (16*pp + t).
                if half in (None, 0):
                    st[p]["qn"] = qn_pool.tile(
                        [128, T, HD], F32, tag="qn", name="qn"
                    )
                qn = st[p]["qn"]
                xr = x[p].rearrange("(pp j) d -> pp j d", j=16)
                if half is None:
                    nc.sync.dma_start(out=qn[:], in_=xr)
                elif half == 0:
                    nc.sync.dma_start(out=qn[:, 0:8, :], in_=xr[:, 0:8, :])
                else:
                    nc.sync.dma_start(out=qn[:, 8:16, :], in_=xr[:, 8:16, :])

            def cast(p, half=None):
                if half in (None, 0):
                    v = v_pool.tile([128, C, HD + 1], F16, tag="v", name="v")
                    nc.vector.memset(v[:, :, HD : HD + 1], 1.0)
                    st[p]["v"] = v
                v = st[p]["v"]
                if half is None:
                    nc.vector.tensor_copy(v[:, :, 0:HD], st[p]["qn"][:])
                elif half == 0:
                    nc.vector.tensor_copy(v[:, 0:8, 0:HD], st[p]["qn"][:, 0:8, :])
                else:
                    nc.vector.tensor_copy(v[:, 8:16, 0:HD], st[p]["qn"][:, 8:16, :])
                if half in (None, 0):
                    st[p]["qT2"] = qT2_pool.tile([128, S], F16, tag="qT2", name="qT2")
                    st[p]["recips"] = sums_pool.tile([128, T], F32, tag="recips", name="recips")
                    st[p]["outsb"] = out_pool.tile([128, T, HD], F32, tag="outsb", name="outsb")

            def transpose_chunk(p, k, view=None):
                # four [128,64] -> [64,128] transposes staged in a PSUM bank
                # (ringB by default), then copied into the low half of qT2
                v = st[p]["v"]
                qT2 = st[p]["qT2"]
                view = qtp_view if view is None else view
                for i, j in enumerate(range(4 * k, 4 * k + 4)):
                    nc.tensor.transpose(
                        view[:, ts(i, 128)], v[:, j, 0:HD], ident[:]
                    )
                nc.vector.tensor_copy(qT2[0:64, ts(k, 512)], view[:, 0:512])

            def dup_chunk(p, k):
                # duplicate into the upper partition half SBUF->SBUF (4x DVE
                # mode, and it keeps the PSUM staging window short)
                qT2 = st[p]["qT2"]
                nc.vector.tensor_copy(
                    qT2[64:128, ts(k, 512)], qT2[0:64, ts(k, 512)]
                )

            def mm1(p, u, s, rg=None):
                # half-tile unit u: q-tile t = u//2, ki columns (u%2)*1024..,
                # written to ring slot s; rg picks the PE row-group (0: array
                # rows 0-63 fed from qT2[0:64], 1: rows 64-127 from the upper
                # duplicate) so adjacent units can run concurrently
                qT2 = st[p]["qT2"]
                t, half = divmod(u, 2)
                rg = half if rg is None else rg
                dst = ringB[:] if s == 2 else ringA[:, s, :]
                rows = slice(0, 64) if rg == 0 else slice(64, 128)
                lhsT = qT2[rows, ts(t, 128)]
                base = half * 1024
                for j in range(2):
                    nc.tensor.matmul(
                        dst[:, ts(j, 512)],
                        lhsT,
                        qT2[rows, base + j * 512 : base + (j + 1) * 512],
                    )

            def _e32(p):
                return st[p]["ebuf"].rearrange("pp c s -> pp (c s)").rearrange(
                    "pp (u k) -> pp u k", k=1024
                )

            def exp_merged(p, ua, ub):
                # one [128, 2048] exp over ringA slots 0,1 -> E units ua, ub
                e32 = _e32(p)
                if ub == ua + 1:
                    dst = e32[:, ua : ua + 2, :]
                else:
                    dst = bass.AP(
                        tensor=e32.tensor,
                        offset=e32.offset + ua * 1024,
                        ap=[list(e32.ap[0]), [(ub - ua) * 1024, 2], [1, 1024]],
                    )
                nc.scalar.activation(
                    dst,
                    ringA[:],
                    mybir.ActivationFunctionType.Exp,
                    bias=bias_tile[:],
                    scale=SCALE,
                )

            def exp_single(p, u):
                nc.scalar.activation(
                    _e32(p)[:, u, :],
                    ringB[:],
                    mybir.ActivationFunctionType.Exp,
                    bias=bias_tile[:],
                    scale=SCALE,
                )

            def mm2norm(p, t, tail=False):
                ebuf, v = st[p]["ebuf"], st[p]["v"]
                if tail:
                    # 3-way rotation over decoupled tensors so the DVE
                    # normalize of tile t-1 overlaps the matmuls of tile t
                    slot = (att01[0][:], att01[1][:], ringB[:, 0 : HD + 1])[t % 3]
                else:
                    slot = att01[t % 2][:]
                for c in range(C):
                    nc.tensor.matmul(
                        slot,
                        ebuf[:, c, ts(t, 128)],
                        v[:, c, :],
                        start=(c == 0),
                        stop=(c == C - 1),
                    )
                rc = st[p]["recips"]
                nc.vector.reciprocal(rc[:, t : t + 1], slot[:, HD : HD + 1])
                nc.vector.tensor_scalar_mul(
                    st[p]["outsb"][:, t, :], slot[:, 0:HD], rc[:, t : t + 1]
                )

            def dma_out(p, quarter=None):
                odram = out[p].rearrange("(pp t) d -> pp t d", t=16)
                if quarter is None:
                    nc.sync.dma_start(out=odram, in_=st[p]["outsb"][:])
                else:
                    q4 = ts(quarter, 4)
                    nc.sync.dma_start(
                        out=odram[:, q4, :], in_=st[p]["outsb"][:, q4, :]
                    )

            def alloc_e(p):
                st[p]["ebuf"] = e_pool.tile([128, C, S], F16, tag="ebuf", name="ebuf")

            def prologue_step(p, h):
                """Spread pair-(p) prologue work across the previous pair's
                unit steps h (called with h = 0..U-1 of the previous pair).
                Transpose chunks are emitted right after a merged exp (h%3==1)
                so the slot-2 borrow lands in slot 2's idle window."""
                if h == 0:
                    dma_in(p)
                elif h == 2:
                    cast(p)
                elif h in (12, 16, 20, 24):
                    transpose_chunk(p, (h - 12) // 4)
                    dup_chunk(p, (h - 12) // 4)
                    if h == 24:
                        alloc_e(p)

            def emit_body():
                # pair-0 prologue, interleaved: chunks alternate between
                # ringB and ringA staging so transposes overlap copies, and
                # the first two mm1 units slot in as their inputs appear
                ra0 = ringA[0:64, 0, :].bitcast(F16)
                dma_in(0, half=0)
                # HAM warm-up: ~2.5us of dummy matmuls while the DMA is in
                # flight, so the PE array is un-throttled (2.4 GHz) when the
                # transposes and first mm1 units arrive
                for _ in range(40):
                    nc.tensor.matmul(att01[0][:], ident[:], ident[:, 0 : HD + 1])
                cast(0, half=0)
                dma_in(0, half=1)
                transpose_chunk(0, 0)
                transpose_chunk(0, 1, view=ra0)
                alloc_e(0)
                mm1(0, 0, 0, rg=0)   # unit 0 (tile 0, ki 0-1023)
                mm1(0, 2, 1, rg=0)   # unit 2 (tile 1, ki 0-1023)
                exp_merged(0, 0, 2)
                cast(0, half=1)
                transpose_chunk(0, 2)
                transpose_chunk(0, 3)
                dma_in(1)  # pair-1 prologue_step(i=0) equivalent
                for k in range(4):
                    dup_chunk(0, k)

                # unit processing order per pair: pair 0 and the last pair run
                # all even units (ki columns 0-1023 of every tile) before the
                # odd ones.  For pair 0 that lets compute start after only two
                # transpose chunks; for the last pair it lets the tail's mm2
                # for q-tiles 0-7 (which reads only even units) start while
                # the odd units are still being exp'd.
                orders = [list(range(U)) for _ in range(PAIRS)]
                orders[0] = list(range(0, U, 2)) + list(range(1, U, 2))
                orders[PAIRS - 1] = list(range(0, U, 2)) + list(range(1, U, 2))

                def aux(p, i):
                    # non-critical work scheduled into unit step i of pair p
                    if p >= 1 and 4 <= i <= 19:
                        mm2norm(p - 1, i - 4)
                        if i == 19:
                            dma_out(p - 1)
                    if p == PAIRS - 1 and 28 <= i <= 31:
                        # emitted strictly after every pair-3 exp, so the
                        # tensor-granular ebuf WAR never gates an exp; ACT's
                        # queued backlog keeps these overlapped on PE
                        mm2norm(p, 2 * (i - 28))
                        mm2norm(p, 2 * (i - 28) + 1)
                        if i in (29, 31):
                            dma_out(p, quarter=(i - 29) // 2)
                    if p + 1 < PAIRS:
                        prologue_step(p + 1, i)

                for p in range(PAIRS):
                    order = orders[p]
                    for i in range(2 if p == 0 else 0, U):
                        mm1(p, order[i], i % 3, rg=i % 2)
                        if i % 3 == 1:
                            exp_merged(p, order[i - 1], order[i])
                            aux(p, i - 1)
                            aux(p, i)
                        elif i % 3 == 2:
                            exp_single(p, order[i])
                            aux(p, i)

                p = PAIRS - 1
                for t in range(8, T):
                    mm2norm(p, t, tail=True)
                    if t % 4 == 3:
                        dma_out(p, quarter=t // 4)

            if loop_reps > 1:
                with tc.For_i(0, loop_reps, 1):
                    emit_body()
            else:
                emit_body()

    _split_sync_waits(nc)
    return nc


_NC_CACHE = None


def kernel(x: np.ndarray) -> np.ndarray:
    global _NC_CACHE
    if _NC_CACHE is None:
        _NC_CACHE = build_kernel()
    nc = _NC_CACHE

    x = np.asarray(x, dtype=np.float32)
    xr = np.reshape(x, (H * B, S, HD))  # flat-buffer reinterpret: pair = h*B + b
    in_maps = [
        {"x": np.ascontiguousarray(xr[i * PAIRS : (i + 1) * PAIRS])}
        for i in range(N_CORES)
    ]
    res = run_bass_kernel_spmd(nc, in_maps, core_ids=list(range(N_CORES)))
    att = np.concatenate([res.results[i]["out"] for i in range(N_CORES)], axis=0)
    att = att.reshape(H, B, S, HD).swapaxes(1, 2).reshape(-1, E_DIM)
    return np.ascontiguousarray(att.astype(np.float32))
